# revision 1
# baseline (speedup 1.0000x reference)
"""Trainium2 Bass kernel for nn_ESBN_77352361001553 (scatter_memory).

Math (see the reference's faithfulness note): the conv encoder is dead code
and the LSTM input is constant zeros, so every batch row follows the same
16-step, 512-dim LSTM trajectory from zero state; the (16, 1024, 4) output
is out_t = Wo @ h_t + bo broadcast across batch. Each of the 8 cores runs
the identical recurrence on-chip and emits its own 128-wide batch shard.

Structure (TileContext; ~65 us vs the 73 us baseline):
 - The gate bias is injected by an N=4 matmul (lhsT = bias rows,
   rhs = I4) that opens each gate group's PSUM accumulation, so the gate
   activations read PSUM directly - no DVE bias-add on the serial tail.
 - Gate order i,g,f,o: the cx chain (t1 = si*tg, cxa = sf*cx, cx, th)
   overlaps the f/o-group matmuls; only sigma_o -> h-mul trails the
   stream. The o-group's first 12 matmuls are emitted interleaved ahead
   of its last 4 so their per-MM semaphore increments (~34 ns each,
   serialized) drain during the stream instead of delaying sigma_o.
 - Gate PSUM banks are double-buffered by step parity (8 banks total;
   the head reuses bank A0 at the end), so the next step's bias matmul
   never waits for this step's gate ACT reads.
 - Weights fp8e4 (x64, descale fused into the activations' free affine:
   sigmoid/tanh(psum/64)), halving the weight DMA to 1 MB across two
   parallel HWDGE queues (SP + Activation); PE warm-up matmuls and the
   ACT sigmoid/tanh table preload run under the DMA.
"""

import os

import numpy as np

T = 16
HID = 512
N_CORES = 8
BSH = 128  # batch shard per core

_BUILT = {}
last_results = None  # BassKernelResults of the most recent run (for tooling)


def _ensure_ntff_hook():
    """Register the axon NTFF profiling hook if the container lacks
    antenv.axon_hooks (slim boot)."""
    import contextlib
    import ctypes
    import sys
    import types

    try:
        from antenv.axon_hooks import get_axon_ntff_profile_hook  # noqa: F401

        return
    except ImportError:
        pass

    so_path = "/opt/axon/libaxon_pjrt.so"
    hook = None
    if os.path.exists(so_path):
        lib = ctypes.CDLL(so_path)
        if hasattr(lib, "axon_start_nrt_profile"):
            lib.axon_start_nrt_profile.argtypes = [
                ctypes.POINTER(ctypes.c_int64),
                ctypes.c_size_t,
            ]
            lib.axon_start_nrt_profile.restype = ctypes.c_int64
            lib.axon_stop_nrt_profile.argtypes = [ctypes.c_char_p]
            lib.axon_stop_nrt_profile.restype = ctypes.c_int64

            @contextlib.contextmanager
            def _hook(output_dir, device_ids):
                import jax

                jax.devices()
                if device_ids:
                    ids = (ctypes.c_int64 * len(device_ids))(*device_ids)
                    rc = lib.axon_start_nrt_profile(ids, len(device_ids))
                else:
                    rc = lib.axon_start_nrt_profile(None, 0)
                if rc != 0:
                    raise RuntimeError(f"axon_start_nrt_profile rc={rc}")
                try:
                    yield
                finally:
                    n = lib.axon_stop_nrt_profile(str(output_dir).encode())
                    print(f"ntff profile: {n} file(s) -> {output_dir}", file=sys.stderr)

            hook = _hook

    mod = types.ModuleType("antenv.axon_hooks")
    mod.get_axon_ntff_profile_hook = lambda: hook
    mod.set_axon_ntff_profile_hook = lambda h: None
    import antenv

    antenv.axon_hooks = mod
    sys.modules["antenv.axon_hooks"] = mod


def _build(nsteps=T):
    import concourse.bacc as bacc
    import concourse.bass as bass
    import concourse.mybir as mybir
    from concourse import tile

    f32 = mybir.dt.float32
    f16 = mybir.dt.float16
    f8 = mybir.dt.float8e4
    AF = mybir.ActivationFunctionType

    nc = bacc.Bacc("TRN2", target_bir_lowering=False, debug=False, enable_asserts=False)

    wT_d = nc.dram_tensor("wT", [128, 8192], f8, kind="ExternalInput")
    aux_d = nc.dram_tensor("aux", [128, 532], f16, kind="ExternalInput")
    bo_d = nc.dram_tensor("bo16", [16, 4], f32, kind="ExternalInput")
    out_d = nc.dram_tensor("out", [T, BSH, 4], f32, kind="ExternalOutput")

    with tile.TileContext(nc) as tc:
        with (
            tc.tile_pool(name="w", bufs=1) as wp,
            tc.tile_pool(name="st", bufs=1) as sp,
            tc.tile_pool(name="tmp", bufs=8) as tp,
            tc.tile_pool(name="ps", bufs=1, space="PSUM") as pp,
            tc.tile_pool(name="psd", bufs=1, space="PSUM") as pdp,
        ):
            wT = wp.tile([128, 8192], f8)
            aux = wp.tile([128, 532], f16)
            bo16 = sp.tile([16, 4], f32)
            id4 = aux[:, 512:516]

            # preload both ACT function tables while the DMAs stream in
            warm = tp.tile([1, 1], f32, tag="warm")
            nc.vector.memset(warm[:], 0.0)
            warm2 = tp.tile([1, 1], f32, tag="warm2")
            nc.scalar.activation(warm2[:], warm[:], AF.Sigmoid)
            nc.scalar.activation(warm2[:], warm[:], AF.Tanh)

            # weight DMA on two HWDGE queues (SP + Activation)
            nc.sync.dma_start(aux[:], aux_d[:])
            nc.sync.dma_start(wT[:, 0:2048], wT_d[:, 0:2048])    # group i
            nc.scalar.dma_start(wT[:, 2048:4096], wT_d[:, 2048:4096])  # g
            nc.sync.dma_start(wT[:, 4096:6144], wT_d[:, 4096:6144])    # f
            nc.scalar.dma_start(wT[:, 6144:8192], wT_d[:, 6144:8192])  # o
            nc.sync.dma_start(bo16[:], bo_d[:])

            # PE warm-up under the DMA: keeps HAM un-throttled into step 1
            wz = sp.tile([128, 128], f16)
            nc.vector.memset(wz[:], 0.0)

            # h history: col 4t+ko holds h_t[ko*128 + p]
            hs = sp.tile([128, 4 * T], f16)
            cx = sp.tile([128, 4], f32)

            # per-group PSUM gates [128, 4] (col c = units c*128+m), double-
            # buffered by step parity so the next step's bias matmul never
            # waits for this step's gate ACT to finish reading (no PSUM WAR
            # stall). 8 tiles = all 8 banks; the head reuses bank A0.
            psgAB = [
                [pp.tile([128, 4], f32, tag=f"ps{k}{n}", name=f"psg{k}{n}") for n in range(4)]
                for k in range(2)
            ]
            # warm-up writes bank B0 (first used at step 1, opened by start=True)
            for _ in range(32):
                nc.tensor.matmul(psgAB[1][0][:], wz[:, 0:128], wz[:, 0:4],
                                 start=True, stop=True, skip_group_check=True)

            def wtile(gi, c, ko):
                i0 = ((4 * gi + c) * 4 + ko) * 128
                return wT[:, i0 : i0 + 128]

            def gates_mm(t, gi, tail=0):
                # bias opens the group (start=True); with tail>0 only the LAST
                # `tail` W-matmuls are emitted here (the rest were emitted
                # interleaved earlier via gates_mm_head)
                psg = psgAB[t % 2]
                nc.tensor.matmul(
                    psg[gi][:, 0:4], aux[:, gi * 128 : gi * 128 + 128], id4,
                    start=True, stop=(t == 0), skip_group_check=True,
                ) if tail == 0 or tail == 16 else None
                if t == 0:
                    return
                for c in range(4):
                    for ko in range(4):
                        idx = c * 4 + ko
                        if tail and idx < 16 - tail:
                            continue
                        nc.tensor.matmul(
                            psg[gi][:, c : c + 1],
                            wtile(gi, c, ko),
                            hs[:, 4 * (t - 1) + ko : 4 * (t - 1) + ko + 1],
                            start=False,
                            stop=(c == 3 and ko == 3),
                            skip_group_check=True,
                        )

            def gates_mm_part(t, gi, first):
                # bias + first `first` W-matmuls of group gi
                psg = psgAB[t % 2]
                nc.tensor.matmul(
                    psg[gi][:, 0:4], aux[:, gi * 128 : gi * 128 + 128], id4,
                    start=True, stop=False, skip_group_check=True,
                )
                for c in range(4):
                    for ko in range(4):
                        if c * 4 + ko >= first:
                            return
                        nc.tensor.matmul(
                            psg[gi][:, c : c + 1],
                            wtile(gi, c, ko),
                            hs[:, 4 * (t - 1) + ko : 4 * (t - 1) + ko + 1],
                            start=False, stop=False, skip_group_check=True,
                        )

            def step(t):
                si = tp.tile([128, 4], f16, tag="si")
                tg = tp.tile([128, 4], f16, tag="tg")
                sf = tp.tile([128, 4], f16, tag="sf")
                so = tp.tile([128, 4], f16, tag="so")
                th = tp.tile([128, 4], f16, tag="th")
                t1 = tp.tile([128, 4], f32, tag="t1")
                cxa = tp.tile([128, 4], f32, tag="cxa")

                psg = psgAB[t % 2]
                gates_mm(t, 0)
                nc.scalar.activation(si[:], psg[0][:, 0:4], AF.Sigmoid, scale=1.0 / 64.0)
                gates_mm(t, 1)
                nc.scalar.activation(tg[:], psg[1][:, 0:4], AF.Tanh, scale=1.0 / 64.0)
                gates_mm(t, 2)
                nc.scalar.activation(sf[:], psg[2][:, 0:4], AF.Sigmoid, scale=1.0 / 64.0)
                if t == 0:
                    gates_mm(t, 3)
                else:
                    # issue o's bias + first 12 W-matmuls interleaved before
                    # f's activation consumers, leaving only 4 W-matmuls (and
                    # their sem increments) on the tail
                    gates_mm_part(t, 3, 12)
                    gates_mm(t, 3, tail=4)
                nc.vector.tensor_mul(t1[:], si[:], tg[:])
                if t == 0:
                    nc.vector.tensor_copy(cx[:], t1[:])
                else:
                    nc.vector.tensor_mul(cxa[:], sf[:], cx[:])
                    nc.vector.tensor_add(cx[:], cxa[:], t1[:])
                nc.scalar.activation(th[:], cx[:], AF.Tanh)
                nc.scalar.activation(so[:], psg[3][:, 0:4], AF.Sigmoid, scale=1.0 / 64.0)
                nc.vector.tensor_mul(hs[:, 4 * t : 4 * t + 4], so[:], th[:])

            for t in range(nsteps):
                step(t)

            # head: hps[t, d] = sum_k Wo[d, k] h_t[k] (+ bo); bank A0 is free
            # after step 14 (set A), so its first 16 partitions hold the head
            hps = psgAB[0][0][0:16, 0:4]
            for ko in range(4):
                nc.tensor.matmul(
                    hps,
                    hs[:, ko : ko + 4 * (T - 1) + 1 : 4],  # lhsT [K=128, M=16]
                    aux[:, 516 + 4 * ko : 520 + 4 * ko],  # rhs [K=128, N=4]
                    start=(ko == 0),
                    stop=(ko == 3),
                )
            head = sp.tile([16, 4], f32)
            nc.vector.tensor_add(head[:], hps, bo16[:])
            # broadcast on-chip to [16, 512] so the output DMA writes 16
            # contiguous 2 KB packets
            bc = sp.tile([16, 512], f32)
            hap = head[:]
            rep = bass.AP(hap.tensor, hap.offset, [list(hap.ap[0]), [0, BSH], [1, 4]])
            nc.vector.tensor_copy(bc[:].rearrange("t (b d) -> t b d", d=4), rep)
            nc.sync.dma_start(out_d.rearrange("t b d -> t (b d)"), bc[:])
    nc.compile()
    return nc


def prep_inputs(Whh, bih, bhh, Wo, bo):
    """Host-side weight relayout (all tensors are tiny: <5 MB total)."""
    Whh = np.asarray(Whh, np.float64)
    c = np.asarray(bih, np.float64) + np.asarray(bhh, np.float64)
    Wo = np.asarray(Wo, np.float32)
    bo = np.asarray(bo, np.float32)
    H = HID
    # gate order i, g, f, o (torch rows: i, f, g, o)
    perm = np.concatenate(
        [
            np.arange(0, H),
            np.arange(2 * H, 3 * H),
            np.arange(H, 2 * H),
            np.arange(3 * H, 4 * H),
        ]
    )
    Wp = (Whh[perm] * 64.0).astype(np.float32)
    cp = (c[perm] * 64.0).astype(np.float32)
    # tile-major interleave: tile (jo, ko) at cols (jo*4+ko)*128,
    # value wT[p, .*128+m] = W_perm[jo*128+m, ko*128+p]
    import ml_dtypes
    wT = np.ascontiguousarray(
        Wp.reshape(16, 128, 4, 128).transpose(3, 0, 2, 1).reshape(128, 8192)
    ).astype(ml_dtypes.float8_e4m3)
    # bias tiles: row k holds the biases of psum column k of the group
    wbias = np.zeros((128, 512), np.float32)
    cpr = cp.reshape(4, 4, 128)  # [gi, k, m]
    for gi in range(4):
        wbias[0:4, gi * 128 : (gi + 1) * 128] = cpr[gi]
    id4 = np.zeros((128, 4), np.float32)
    id4[np.arange(4), np.arange(4)] = 1.0
    woT = np.ascontiguousarray(
        Wo.reshape(4, 4, 128).transpose(2, 1, 0).reshape(128, 16)
    )
    aux = np.concatenate([wbias, id4, woT], axis=1).astype(np.float16)  # [128, 532]
    bo16 = np.tile(bo, (16, 1)).astype(np.float32)
    return {"wT": wT, "aux": aux, "bo16": bo16}


def kernel(**inputs) -> np.ndarray:
    global last_results
    from concourse.bass_utils import run_bass_kernel_spmd

    if "nc" not in _BUILT:
        _BUILT["nc"] = _build()
    nc = _BUILT["nc"]

    in_map = prep_inputs(
        inputs["Whh"], inputs["bih"], inputs["bhh"], inputs["Wo"], inputs["bo"]
    )
    if os.environ.get("BASS_TRACE"):
        _ensure_ntff_hook()
    in_maps = [dict(in_map) for _ in range(N_CORES)]
    res = run_bass_kernel_spmd(
        nc,
        in_maps,
        core_ids=list(range(N_CORES)),
        trace=bool(os.environ.get("BASS_TRACE")),
    )
    last_results = res
    return np.concatenate([r["out"] for r in res.results], axis=1)



# revision 7
# speedup vs baseline: 1.0742x; 1.0742x over previous
"""Trainium2 Bass kernel for nn_ESBN_77352361001553 (scatter_memory).

Math (see the reference's faithfulness note): the conv encoder is dead code
and the LSTM input is constant zeros, so every batch row follows the same
16-step, 512-dim LSTM trajectory from zero state; the (16, 1024, 4) output
is out_t = Wo @ h_t + bo broadcast across batch. Each of the 8 cores runs
the identical recurrence on-chip and emits its own 128-wide batch shard.

This version is raw Bass (no TileContext) with hand-placed semaphores:
 - One then_inc per 16-matmul gate group instead of one per matmul. The
   tile framework's per-MM increments serialize at ~34 ns on the EVT_SEM
   port while the LDW+MM pairs stream at 27 ns, building a ~450 ns
   semaphore backlog that delayed every gate activation by that much.
 - One PE wait per step: the group-i bias matmul waits for h_{t-1}. All
   other PSUM/tile hazards are transitively ordered by engine FIFOs
   (verified by hand; the bias matmul also shields the W-matmul
   LDWEIGHTS from the PE's pull-ahead window since its own stationary
   is the constant aux tile).
 - Gate banks: group gi at step t lives in PSUM bank 4*(t%2)+gi, so the
   scalar engine never reads a bank the PE is writing (collision-fatal).
 - Weights fp8e4 (x64, descale fused into the activation scale), biases
   injected per group by an N=4 matmul (lhsT = bias rows, rhs = I4).
"""

import os

import numpy as np

T = 16
HID = 512
N_CORES = 8
BSH = 128  # batch shard per core

_BUILT = {}
last_results = None  # BassKernelResults of the most recent run (for tooling)


def _ensure_ntff_hook():
    """Register the axon NTFF profiling hook if the container lacks
    antenv.axon_hooks (slim boot)."""
    import contextlib
    import ctypes
    import sys
    import types

    try:
        from antenv.axon_hooks import get_axon_ntff_profile_hook  # noqa: F401

        return
    except ImportError:
        pass

    so_path = "/opt/axon/libaxon_pjrt.so"
    hook = None
    if os.path.exists(so_path):
        lib = ctypes.CDLL(so_path)
        if hasattr(lib, "axon_start_nrt_profile"):
            lib.axon_start_nrt_profile.argtypes = [
                ctypes.POINTER(ctypes.c_int64),
                ctypes.c_size_t,
            ]
            lib.axon_start_nrt_profile.restype = ctypes.c_int64
            lib.axon_stop_nrt_profile.argtypes = [ctypes.c_char_p]
            lib.axon_stop_nrt_profile.restype = ctypes.c_int64

            @contextlib.contextmanager
            def _hook(output_dir, device_ids):
                import jax

                jax.devices()
                if device_ids:
                    ids = (ctypes.c_int64 * len(device_ids))(*device_ids)
                    rc = lib.axon_start_nrt_profile(ids, len(device_ids))
                else:
                    rc = lib.axon_start_nrt_profile(None, 0)
                if rc != 0:
                    raise RuntimeError(f"axon_start_nrt_profile rc={rc}")
                try:
                    yield
                finally:
                    n = lib.axon_stop_nrt_profile(str(output_dir).encode())
                    print(f"ntff profile: {n} file(s) -> {output_dir}", file=sys.stderr)

            hook = _hook

    mod = types.ModuleType("antenv.axon_hooks")
    mod.get_axon_ntff_profile_hook = lambda: hook
    mod.set_axon_ntff_profile_hook = lambda h: None
    import antenv

    antenv.axon_hooks = mod
    sys.modules["antenv.axon_hooks"] = mod


def _build():
    import concourse.bacc as bacc
    import concourse.bass as bass
    import concourse.mybir as mybir

    f32 = mybir.dt.float32
    f16 = mybir.dt.float16
    f8 = mybir.dt.float8e4
    AF = mybir.ActivationFunctionType

    nc = bacc.Bacc("TRN2", target_bir_lowering=False, debug=False, enable_asserts=False)

    wT_d = nc.dram_tensor("wT", [128, 8192], f8, kind="ExternalInput")
    aux_d = nc.dram_tensor("aux", [128, 532], f16, kind="ExternalInput")
    bo_d = nc.dram_tensor("bo16", [16, 4], f32, kind="ExternalInput")
    out_d = nc.dram_tensor("out", [T, BSH, 4], f32, kind="ExternalOutput")

    # SBUF (persistent allocations; no pools needed for a fixed kernel)
    wT = nc.alloc_sbuf_tensor("wTs", [128, 8192], f8)
    aux = nc.alloc_sbuf_tensor("auxs", [128, 532], f16)
    bo16 = nc.alloc_sbuf_tensor("bo16s", [16, 4], f32)
    hs = nc.alloc_sbuf_tensor("hss", [128, 4 * T], f16)
    cx = nc.alloc_sbuf_tensor("cxs", [128, 4], f32)
    si = nc.alloc_sbuf_tensor("sis", [128, 4], f16)
    tg = nc.alloc_sbuf_tensor("tgs", [128, 4], f16)
    sf = nc.alloc_sbuf_tensor("sfs", [128, 4], f16)
    so = nc.alloc_sbuf_tensor("sos", [128, 4], f16)
    th = nc.alloc_sbuf_tensor("ths", [128, 4], f16)
    t1 = nc.alloc_sbuf_tensor("t1s", [128, 4], f32)
    cxa = nc.alloc_sbuf_tensor("cxas", [128, 4], f32)
    wz = nc.alloc_sbuf_tensor("wzs", [128, 128], f16)  # warmup lhsT (garbage ok)
    w2 = nc.alloc_sbuf_tensor("w2s", [128, 4], f16)  # ACT-table warm scratch
    head = nc.alloc_sbuf_tensor("heads", [16, 4], f32)
    bc = nc.alloc_sbuf_tensor("bcs", [16, 512], f32)

    id4 = aux[:, 512:516]

    # PSUM: all 8 banks; group gi at step t -> bank 4*(t%2)+gi, cols 0:4.
    pb = [nc.alloc_psum_tensor(f"pb{i}", [128, 512], f32) for i in range(8)]

    def bank(t, gi):
        return pb[4 * (t % 2) + gi]

    s_aux = nc.alloc_semaphore("s_aux")  # aux DMA completion
    s_dma_a = nc.alloc_semaphore("s_dma_a")  # SP HWDGE queue completions
    s_dma_b = nc.alloc_semaphore("s_dma_b")  # Act HWDGE queue completions
    s_pe = nc.alloc_semaphore("s_pe")  # PE gate-group completions
    s_act = nc.alloc_semaphore("s_act")  # scalar ACT completions
    s_dve = nc.alloc_semaphore("s_dve")  # vector completions
    s_out = nc.alloc_semaphore("s_out")  # output DMA
    all_sems = [s_aux, s_dma_a, s_dma_b, s_pe, s_act, s_dve, s_out]

    # ---- static tick tables ------------------------------------------------
    # s_pe: t=0 emits bias i,g,o (f skipped: cx_0 = i*g); t>=1: 4 groups.
    pe_tick = {}
    n = 0
    for gi in (0, 1, 3):
        n += 1
        pe_tick[(0, gi)] = n
    for t in range(1, T):
        for gi in range(4):
            n += 1
            pe_tick[(t, gi)] = n
    n += 1
    pe_tick["head"] = n

    # s_act: per step sigma_i, tanh_g, [sigma_f], sigma_o, tanh_cx
    act_tick = {}
    n = 0
    for t in range(T):
        for key in ("i", "g", "f", "o", "h"):
            if t == 0 and key == "f":
                continue
            n += 1
            act_tick[(t, key)] = n

    # s_dve: t=0: cx, h; t>=1: cxa, cx, h; head: add, bc
    dve_tick = {}
    n = 0
    for t in range(T):
        for key in (("cx", "h") if t == 0 else ("cxa", "cx", "h")):
            n += 1
            dve_tick[(t, key)] = n
    n += 1
    dve_tick["headadd"] = n
    n += 1
    dve_tick["bc"] = n

    def wtile(gi, c, ko):
        i0 = ((4 * gi + c) * 4 + ko) * 128
        return wT[:, i0 : i0 + 128]

    # ---- block 0: clear semaphores (kernel must be re-runnable) -----------
    with nc.Block("clr") as blk0:

        @blk0.sync
        def _(sync):
            for s in all_sems:
                sync.sem_clear(s)

        @blk0.vector
        def _(vector):
            vector.memset(wz[:], 0.0)

    # ---- main block --------------------------------------------------------
    with nc.Block("main") as blk:

        @blk.sync
        def _(sync):
            sync.dma_start(aux[:], aux_d[:]).then_inc(s_aux, 16)
            sync.dma_start(wT[:, 0:4096], wT_d[:, 0:4096]).then_inc(s_dma_a, 16)
            sync.dma_start(bo16[:], bo_d[:]).then_inc(s_dma_a, 16)
            sync.wait_ge(s_dve, dve_tick["bc"])
            sync.dma_start(
                out_d.rearrange("t b d -> t (b d)"), bc[:]
            ).then_inc(s_out, 16)
            sync.wait_ge(s_out, 16)

        @blk.scalar
        def _(scalar):
            scalar.dma_start(wT[:, 4096:8192], wT_d[:, 4096:8192]).then_inc(
                s_dma_b, 16
            )
            # preload both ACT tables (input garbage is fine; output scratch)
            scalar.activation(w2[:, 0:2], wz[:, 0:2], AF.Sigmoid)
            scalar.activation(w2[:, 2:4], wz[:, 2:4], AF.Tanh)
            for t in range(T):
                scalar.activation(
                    si[:], bank(t, 0)[:, 0:4], AF.Sigmoid, scale=1.0 / 64.0
                )._wait_ge(s_pe, pe_tick[(t, 0)]).then_inc(s_act)
                scalar.activation(
                    tg[:], bank(t, 1)[:, 0:4], AF.Tanh, scale=1.0 / 64.0
                )._wait_ge(s_pe, pe_tick[(t, 1)]).then_inc(s_act)
                if t > 0:
                    scalar.activation(
                        sf[:], bank(t, 2)[:, 0:4], AF.Sigmoid, scale=1.0 / 64.0
                    )._wait_ge(s_pe, pe_tick[(t, 2)]).then_inc(s_act)
                scalar.activation(
                    so[:], bank(t, 3)[:, 0:4], AF.Sigmoid, scale=1.0 / 64.0
                )._wait_ge(s_pe, pe_tick[(t, 3)]).then_inc(s_act)
                scalar.activation(th[:], cx[:], AF.Tanh)._wait_ge(
                    s_dve, dve_tick[(t, "cx")]
                ).then_inc(s_act)

        @blk.tensor
        def _(tensor):
            def warm(k, first_start=False):
                # start=False: never clears a bank (bank 3 col 16 is scratch);
                # only the very first warmup MM opens the bank.
                for j in range(k):
                    tensor.matmul(
                        pb[3][:, 16:17],
                        wz[:],
                        wz[:, 0:1],
                        start=(first_start and j == 0),
                        stop=False,
                        skip_group_check=True,
                    )

            def bias_mm(t, gi, stop):
                return tensor.matmul(
                    bank(t, gi)[:, 0:4],
                    aux[:, gi * 128 : gi * 128 + 128],
                    id4,
                    start=True,
                    stop=stop,
                    skip_group_check=True,
                )

            warm(30, first_start=True)
            # step 0: gates are pure bias (h_{-1} = 0); group f unused
            bias_mm(0, 0, True)._wait_ge(s_aux, 16).then_inc(s_pe)
            bias_mm(0, 1, True).then_inc(s_pe)
            bias_mm(0, 3, True).then_inc(s_pe)
            warm(16)
            for t in range(1, T):
                if t == 1:
                    tensor.wait_ge(s_dma_a, 32)  # wT first half + bo16
                    tensor.wait_ge(s_dma_b, 16)
                for gi in range(4):
                    b = bias_mm(t, gi, False)
                    if gi == 0:
                        b._wait_ge(s_dve, dve_tick[(t - 1, "h")])
                    for c in range(4):
                        for ko in range(4):
                            mm = tensor.matmul(
                                bank(t, gi)[:, c : c + 1],
                                wtile(gi, c, ko),
                                hs[:, 4 * (t - 1) + ko : 4 * (t - 1) + ko + 1],
                                start=False,
                                stop=(c == 3 and ko == 3),
                                skip_group_check=True,
                            )
                    mm.then_inc(s_pe)
            # dummy MM carries the h_15 wait so the head matmuls' LDWEIGHTS
            # (which read hs) cannot be pulled ahead of it
            tensor.matmul(
                pb[3][:, 16:17],
                wz[:],
                wz[:, 0:1],
                start=False,
                stop=False,
                skip_group_check=True,
            )._wait_ge(s_dve, dve_tick[(T - 1, "h")])
            for ko in range(4):
                hm = tensor.matmul(
                    pb[3][0:16, 20:24],
                    hs[:, ko : ko + 4 * (T - 1) + 1 : 4],
                    aux[:, 516 + 4 * ko : 520 + 4 * ko],
                    start=(ko == 0),
                    stop=(ko == 3),
                    skip_group_check=True,
                )
            hm.then_inc(s_pe)

        @blk.vector
        def _(vector):
            for t in range(T):
                if t == 0:
                    vector.tensor_mul(cx[:], si[:], tg[:])._wait_ge(
                        s_act, act_tick[(0, "g")]
                    ).then_inc(s_dve)
                else:
                    vector.tensor_mul(t1[:], si[:], tg[:])._wait_ge(
                        s_act, act_tick[(t, "g")]
                    )
                    vector.tensor_mul(cxa[:], sf[:], cx[:])._wait_ge(
                        s_act, act_tick[(t, "f")]
                    ).then_inc(s_dve)
                    vector.tensor_add(cx[:], cxa[:], t1[:])._wait_ge(
                        s_dve, dve_tick[(t, "cxa")]
                    ).then_inc(s_dve)
                vector.tensor_mul(
                    hs[:, 4 * t : 4 * t + 4], so[:], th[:]
                )._wait_ge(s_act, act_tick[(t, "h")]).then_inc(s_dve)
            vector.wait_ge(s_dma_a, 32)
            vector.tensor_add(head[:], pb[3][0:16, 20:24], bo16[:])._wait_ge(
                s_pe, pe_tick["head"]
            ).then_inc(s_dve)
            hap = head[:]
            rep = bass.AP(hap.tensor, hap.offset, [list(hap.ap[0]), [0, BSH], [1, 4]])
            vector.tensor_copy(
                bc[:].rearrange("t (b d) -> t b d", d=4), rep
            )._wait_ge(s_dve, dve_tick["headadd"]).then_inc(s_dve)

    nc.compile()
    return nc


def prep_inputs(Whh, bih, bhh, Wo, bo):
    """Host-side weight relayout (all tensors are tiny: <5 MB total)."""
    Whh = np.asarray(Whh, np.float64)
    c = np.asarray(bih, np.float64) + np.asarray(bhh, np.float64)
    Wo = np.asarray(Wo, np.float32)
    bo = np.asarray(bo, np.float32)
    H = HID
    # gate order i, g, f, o (torch rows: i, f, g, o)
    perm = np.concatenate(
        [
            np.arange(0, H),
            np.arange(2 * H, 3 * H),
            np.arange(H, 2 * H),
            np.arange(3 * H, 4 * H),
        ]
    )
    Wp = (Whh[perm] * 64.0).astype(np.float32)
    cp = (c[perm] * 64.0).astype(np.float32)
    # tile-major interleave: tile (jo, ko) at cols (jo*4+ko)*128,
    # value wT[p, .*128+m] = W_perm[jo*128+m, ko*128+p]
    import ml_dtypes

    wT = np.ascontiguousarray(
        Wp.reshape(16, 128, 4, 128).transpose(3, 0, 2, 1).reshape(128, 8192)
    ).astype(ml_dtypes.float8_e4m3)
    # bias tiles: row k holds the biases of psum column k of the group
    wbias = np.zeros((128, 512), np.float32)
    cpr = cp.reshape(4, 4, 128)  # [gi, k, m]
    for gi in range(4):
        wbias[0:4, gi * 128 : (gi + 1) * 128] = cpr[gi]
    id4 = np.zeros((128, 4), np.float32)
    id4[np.arange(4), np.arange(4)] = 1.0
    woT = np.ascontiguousarray(
        Wo.reshape(4, 4, 128).transpose(2, 1, 0).reshape(128, 16)
    )
    aux = np.concatenate([wbias, id4, woT], axis=1).astype(np.float16)  # [128, 532]
    bo16 = np.tile(bo, (16, 1)).astype(np.float32)
    return {"wT": wT, "aux": aux, "bo16": bo16}


def kernel(**inputs) -> np.ndarray:
    global last_results
    from concourse.bass_utils import run_bass_kernel_spmd

    if "nc" not in _BUILT:
        _BUILT["nc"] = _build()
    nc = _BUILT["nc"]

    in_map = prep_inputs(
        inputs["Whh"], inputs["bih"], inputs["bhh"], inputs["Wo"], inputs["bo"]
    )
    if os.environ.get("BASS_TRACE"):
        _ensure_ntff_hook()
    in_maps = [dict(in_map) for _ in range(N_CORES)]
    res = run_bass_kernel_spmd(
        nc,
        in_maps,
        core_ids=list(range(N_CORES)),
        trace=bool(os.environ.get("BASS_TRACE")),
    )
    last_results = res
    return np.concatenate([r["out"] for r in res.results], axis=1)


# revision 13
# speedup vs baseline: 1.1158x; 1.0387x over previous
"""Trainium2 Bass kernel for nn_ESBN_77352361001553 (scatter_memory).

Math (see the reference's faithfulness note): the conv encoder is dead code
and the LSTM input is constant zeros, so every batch row follows the same
16-step, 512-dim LSTM trajectory from zero state; the (16, 1024, 4) output
is out_t = Wo @ h_t + bo broadcast across batch. Each of the 8 cores runs
the identical recurrence on-chip and emits its own 128-wide batch shard.

This version is raw Bass (no TileContext) with hand-placed semaphores:
 - One then_inc per 16-matmul gate group instead of one per matmul. The
   tile framework's per-MM increments serialize at ~34 ns on the EVT_SEM
   port while the LDW+MM pairs stream at 27 ns, building a ~450 ns
   semaphore backlog that delayed every gate activation by that much.
 - One PE wait per step: the group-i bias matmul waits for h_{t-1}. All
   other PSUM/tile hazards are transitively ordered by engine FIFOs
   (verified by hand; the bias matmul also shields the W-matmul
   LDWEIGHTS from the PE's pull-ahead window since its own stationary
   is the constant aux tile).
 - Gate banks: group gi at step t lives in PSUM bank 4*(t%2)+gi, so the
   scalar engine never reads a bank the PE is writing (collision-fatal).
 - Weights fp8e4 (x64, descale fused into the activation scale), biases
   injected per group by an N=4 matmul (lhsT = bias rows, rhs = I4).
"""

import os

import numpy as np

T = 16
HID = 512
N_CORES = 8
BSH = 128  # batch shard per core

_BUILT = {}
last_results = None  # BassKernelResults of the most recent run (for tooling)


def _ensure_ntff_hook():
    """Register the axon NTFF profiling hook if the container lacks
    antenv.axon_hooks (slim boot)."""
    import contextlib
    import ctypes
    import sys
    import types

    try:
        from antenv.axon_hooks import get_axon_ntff_profile_hook  # noqa: F401

        return
    except ImportError:
        pass

    so_path = "/opt/axon/libaxon_pjrt.so"
    hook = None
    if os.path.exists(so_path):
        lib = ctypes.CDLL(so_path)
        if hasattr(lib, "axon_start_nrt_profile"):
            lib.axon_start_nrt_profile.argtypes = [
                ctypes.POINTER(ctypes.c_int64),
                ctypes.c_size_t,
            ]
            lib.axon_start_nrt_profile.restype = ctypes.c_int64
            lib.axon_stop_nrt_profile.argtypes = [ctypes.c_char_p]
            lib.axon_stop_nrt_profile.restype = ctypes.c_int64

            @contextlib.contextmanager
            def _hook(output_dir, device_ids):
                import jax

                jax.devices()
                if device_ids:
                    ids = (ctypes.c_int64 * len(device_ids))(*device_ids)
                    rc = lib.axon_start_nrt_profile(ids, len(device_ids))
                else:
                    rc = lib.axon_start_nrt_profile(None, 0)
                if rc != 0:
                    raise RuntimeError(f"axon_start_nrt_profile rc={rc}")
                try:
                    yield
                finally:
                    n = lib.axon_stop_nrt_profile(str(output_dir).encode())
                    print(f"ntff profile: {n} file(s) -> {output_dir}", file=sys.stderr)

            hook = _hook

    mod = types.ModuleType("antenv.axon_hooks")
    mod.get_axon_ntff_profile_hook = lambda: hook
    mod.set_axon_ntff_profile_hook = lambda h: None
    import antenv

    antenv.axon_hooks = mod
    sys.modules["antenv.axon_hooks"] = mod


def _build():
    import concourse.bacc as bacc
    import concourse.bass as bass
    import concourse.mybir as mybir

    f32 = mybir.dt.float32
    f16 = mybir.dt.float16
    f8 = mybir.dt.float8e4
    AF = mybir.ActivationFunctionType

    nc = bacc.Bacc("TRN2", target_bir_lowering=False, debug=False, enable_asserts=False)

    wT_d = nc.dram_tensor("wT", [128, 8192], f8, kind="ExternalInput")
    aux_d = nc.dram_tensor("aux", [128, 532], f16, kind="ExternalInput")
    bo_d = nc.dram_tensor("bo16", [16, 4], f32, kind="ExternalInput")
    out_d = nc.dram_tensor("out", [T, BSH, 4], f32, kind="ExternalOutput")

    # SBUF (persistent allocations; no pools needed for a fixed kernel)
    wT = nc.alloc_sbuf_tensor("wTs", [128, 8192], f8)
    aux = nc.alloc_sbuf_tensor("auxs", [128, 532], f16)
    bo16 = nc.alloc_sbuf_tensor("bo16s", [16, 4], f32)
    hs = nc.alloc_sbuf_tensor("hss", [128, 4 * T], f16)
    cx = nc.alloc_sbuf_tensor("cxs", [128, 4], f32)
    si = nc.alloc_sbuf_tensor("sis", [128, 4], f16)
    tg = nc.alloc_sbuf_tensor("tgs", [128, 4], f16)
    sf = nc.alloc_sbuf_tensor("sfs", [128, 4], f16)
    so = nc.alloc_sbuf_tensor("sos", [128, 4], f16)
    th = nc.alloc_sbuf_tensor("ths", [128, 4], f16)
    t1 = nc.alloc_sbuf_tensor("t1s", [128, 4], f32)
    cxa = nc.alloc_sbuf_tensor("cxas", [128, 4], f32)
    wz = nc.alloc_sbuf_tensor("wzs", [128, 128], f16)  # warmup lhsT (garbage ok)
    w2 = nc.alloc_sbuf_tensor("w2s", [128, 4], f16)  # ACT-table warm scratch
    head = nc.alloc_sbuf_tensor("heads", [16, 4], f32)
    bc = nc.alloc_sbuf_tensor("bcs", [16, 512], f32)

    id4 = aux[:, 512:516]

    # PSUM: all 8 banks; group gi at step t -> bank 4*(t%2)+gi, cols 0:4.
    pb = [nc.alloc_psum_tensor(f"pb{i}", [128, 512], f32) for i in range(8)]

    def bank(t, gi):
        return pb[4 * (t % 2) + gi]

    s_dma_a = nc.alloc_semaphore("s_dma_a")  # wT DMA (SP HWDGE queue)
    s_dma_b = nc.alloc_semaphore("s_dma_b")  # aux DMA (Act HWDGE queue)
    s_pe = nc.alloc_semaphore("s_pe")  # PE gate-group completions
    s_act = nc.alloc_semaphore("s_act")  # scalar ACT completions
    s_dve = nc.alloc_semaphore("s_dve")  # vector completions
    s_out = nc.alloc_semaphore("s_out")  # output DMA

    # ---- static tick tables ------------------------------------------------
    # s_pe: t=0 emits bias i,g,o (f skipped: cx_0 = i*g); t>=1: 4 groups.
    pe_tick = {}
    n = 0
    for gi in (0, 1, 3):
        n += 1
        pe_tick[(0, gi)] = n
    for t in range(1, T):
        for gi in range(4):
            n += 1
            pe_tick[(t, gi)] = n
    n += 1
    pe_tick["head"] = n

    # s_act: per step sigma_i, tanh_g, [sigma_f], sigma_o, tanh_cx
    act_tick = {}
    n = 0
    for t in range(T):
        for key in ("i", "g", "f", "o", "h"):
            if t == 0 and key == "f":
                continue
            n += 1
            act_tick[(t, key)] = n

    # s_dve: wz memset; t=0: cx, h; t>=1: cxa, cx, h; head: add, bc
    dve_tick = {}
    n = 1
    dve_tick["wz"] = 1
    for t in range(T):
        for key in (("cx", "h") if t == 0 else ("cxa", "cx", "h")):
            n += 1
            dve_tick[(t, key)] = n
    n += 1
    dve_tick["headadd"] = n
    n += 1
    dve_tick["bc"] = n

    def wtile(gi, c, ko):
        i0 = ((4 * gi + c) * 4 + ko) * 128
        return wT[:, i0 : i0 + 128]

    # ---- main block --------------------------------------------------------
    # (no explicit sem clears: the Bass preamble range-clears the kernel
    # semaphore range on every run)
    with nc.Block("main") as blk:

        @blk.sync
        def _(sync):
            sync.dma_start(wT[:], wT_d[:]).then_inc(s_dma_a, 16)
            sync.wait_ge(s_dve, dve_tick["bc"])
            sync.dma_start(
                out_d.rearrange("t b d -> t (b d)"), bc[:]
            ).then_inc(s_out, 16)
            sync.wait_ge(s_out, 16)

        @blk.scalar
        def _(scalar):
            scalar.dma_start(aux[:], aux_d[:]).then_inc(s_dma_b, 16)
            scalar.dma_start(bo16[:], bo_d[:]).then_inc(s_dma_b, 16)
            # preload both ACT tables (input zeros; output scratch)
            scalar.wait_ge(s_dve, dve_tick["wz"])
            scalar.activation(w2[:, 0:2], wz[:, 0:2], AF.Sigmoid)
            scalar.activation(w2[:, 2:4], wz[:, 2:4], AF.Tanh)
            for t in range(T):
                scalar.activation(
                    si[:], bank(t, 0)[:, 0:4], AF.Sigmoid, scale=1.0 / 64.0
                )._wait_ge(s_pe, pe_tick[(t, 0)]).then_inc(s_act)
                scalar.activation(
                    tg[:], bank(t, 1)[:, 0:4], AF.Tanh, scale=1.0 / 64.0
                )._wait_ge(s_pe, pe_tick[(t, 1)]).then_inc(s_act)
                if t > 0:
                    scalar.activation(
                        sf[:], bank(t, 2)[:, 0:4], AF.Sigmoid, scale=1.0 / 64.0
                    )._wait_ge(s_pe, pe_tick[(t, 2)]).then_inc(s_act)
                scalar.activation(
                    so[:], bank(t, 3)[:, 0:4], AF.Sigmoid, scale=1.0 / 64.0
                )._wait_ge(s_pe, pe_tick[(t, 3)]).then_inc(s_act)
                scalar.activation(
                    bank(t, 0)[:, 8:12], cx[:], AF.Tanh
                )._wait_ge(s_dve, dve_tick[(t, "cx")]).then_inc(s_act)

        @blk.tensor
        def _(tensor):
            def warm(k, first_start=False):
                # start=False: never clears a bank (bank 3 col 16 is scratch);
                # only the very first warmup MM opens the bank.
                for j in range(k):
                    tensor.matmul(
                        pb[3][:, 16:17],
                        wz[:],
                        wz[:, 0:1],
                        start=(first_start and j == 0),
                        stop=False,
                        skip_group_check=True,
                    )

            def bias_mm(t, gi, stop):
                return tensor.matmul(
                    bank(t, gi)[:, 0:4],
                    aux[:, gi * 128 : gi * 128 + 128],
                    id4,
                    start=True,
                    stop=stop,
                    skip_group_check=True,
                )

            tensor.wait_ge(s_dve, dve_tick["wz"])
            # all warmups strictly precede the step-0 bias MMs: a warm MM
            # writing bank 3 while sigma_o(0) reads it would be a fatal
            # PSUM bank collision
            warm(45, first_start=True)
            # step 0: gates are pure bias (h_{-1} = 0); group f unused
            bias_mm(0, 0, True)._wait_ge(s_dma_b, 32).then_inc(s_pe)
            bias_mm(0, 1, True).then_inc(s_pe)
            bias_mm(0, 3, True).then_inc(s_pe)
            for t in range(1, T):
                # all 4 bias MMs first: they depend on nothing recent, so
                # they overlap the previous step's activation tail
                for gi in range(4):
                    bias_mm(t, gi, False)
                if t == 1:
                    tensor.wait_ge(s_dma_a, 16)  # wT
                for gi in range(4):
                    for c in range(4):
                        for ko in range(4):
                            mm = tensor.matmul(
                                bank(t, gi)[:, c : c + 1],
                                wtile(gi, c, ko),
                                hs[:, 4 * (t - 1) + ko : 4 * (t - 1) + ko + 1],
                                start=False,
                                stop=(c == 3 and ko == 3),
                                skip_group_check=True,
                            )
                            if gi == 0 and c == 0 and ko == 0:
                                mm._wait_ge(s_dve, dve_tick[(t - 1, "h")])
                    mm.then_inc(s_pe)
            # dummy MM carries the h_15 wait so the head matmuls' LDWEIGHTS
            # (which read hs) cannot be pulled ahead of it
            tensor.matmul(
                pb[3][:, 16:17],
                wz[:],
                wz[:, 0:1],
                start=False,
                stop=False,
                skip_group_check=True,
            )._wait_ge(s_dve, dve_tick[(T - 1, "h")])
            for ko in range(4):
                hm = tensor.matmul(
                    pb[3][0:16, 20:24],
                    hs[:, ko : ko + 4 * (T - 1) + 1 : 4],
                    aux[:, 516 + 4 * ko : 520 + 4 * ko],
                    start=(ko == 0),
                    stop=(ko == 3),
                    skip_group_check=True,
                )
            hm.then_inc(s_pe)

        @blk.vector
        def _(vector):
            vector.memset(wz[:], 0.0).then_inc(s_dve)
            for t in range(T):
                if t == 0:
                    vector.tensor_mul(cx[:], si[:], tg[:])._wait_ge(
                        s_act, act_tick[(0, "g")]
                    ).then_inc(s_dve)
                else:
                    vector.tensor_mul(t1[:], si[:], tg[:])._wait_ge(
                        s_act, act_tick[(t, "g")]
                    )
                    vector.tensor_mul(cxa[:], sf[:], cx[:])._wait_ge(
                        s_act, act_tick[(t, "f")]
                    ).then_inc(s_dve)
                    vector.tensor_add(cx[:], cxa[:], t1[:])._wait_ge(
                        s_dve, dve_tick[(t, "cxa")]
                    ).then_inc(s_dve)
                vector.tensor_mul(
                    hs[:, 4 * t : 4 * t + 4], so[:], bank(t, 0)[:, 8:12]
                )._wait_ge(s_act, act_tick[(t, "h")]).then_inc(s_dve)
            vector.tensor_add(head[:], pb[3][0:16, 20:24], bo16[:])._wait_ge(
                s_pe, pe_tick["head"]
            ).then_inc(s_dve)
            hap = head[:]
            rep = bass.AP(hap.tensor, hap.offset, [list(hap.ap[0]), [0, BSH], [1, 4]])
            vector.tensor_copy(
                bc[:].rearrange("t (b d) -> t b d", d=4), rep
            )._wait_ge(s_dve, dve_tick["headadd"]).then_inc(s_dve)

    nc.compile()
    return nc


def prep_inputs(Whh, bih, bhh, Wo, bo):
    """Host-side weight relayout (all tensors are tiny: <5 MB total)."""
    Whh = np.asarray(Whh, np.float64)
    c = np.asarray(bih, np.float64) + np.asarray(bhh, np.float64)
    Wo = np.asarray(Wo, np.float32)
    bo = np.asarray(bo, np.float32)
    H = HID
    # gate order i, g, f, o (torch rows: i, f, g, o)
    perm = np.concatenate(
        [
            np.arange(0, H),
            np.arange(2 * H, 3 * H),
            np.arange(H, 2 * H),
            np.arange(3 * H, 4 * H),
        ]
    )
    Wp = (Whh[perm] * 64.0).astype(np.float32)
    cp = (c[perm] * 64.0).astype(np.float32)
    # tile-major interleave: tile (jo, ko) at cols (jo*4+ko)*128,
    # value wT[p, .*128+m] = W_perm[jo*128+m, ko*128+p]
    import ml_dtypes

    wT = np.ascontiguousarray(
        Wp.reshape(16, 128, 4, 128).transpose(3, 0, 2, 1).reshape(128, 8192)
    ).astype(ml_dtypes.float8_e4m3)
    # bias tiles: row k holds the biases of psum column k of the group
    wbias = np.zeros((128, 512), np.float32)
    cpr = cp.reshape(4, 4, 128)  # [gi, k, m]
    for gi in range(4):
        wbias[0:4, gi * 128 : (gi + 1) * 128] = cpr[gi]
    id4 = np.zeros((128, 4), np.float32)
    id4[np.arange(4), np.arange(4)] = 1.0
    woT = np.ascontiguousarray(
        Wo.reshape(4, 4, 128).transpose(2, 1, 0).reshape(128, 16)
    )
    aux = np.concatenate([wbias, id4, woT], axis=1).astype(np.float16)  # [128, 532]
    bo16 = np.tile(bo, (16, 1)).astype(np.float32)
    return {"wT": wT, "aux": aux, "bo16": bo16}


def kernel(**inputs) -> np.ndarray:
    global last_results
    from concourse.bass_utils import run_bass_kernel_spmd

    if "nc" not in _BUILT:
        _BUILT["nc"] = _build()
    nc = _BUILT["nc"]

    in_map = prep_inputs(
        inputs["Whh"], inputs["bih"], inputs["bhh"], inputs["Wo"], inputs["bo"]
    )
    if os.environ.get("BASS_TRACE"):
        _ensure_ntff_hook()
    in_maps = [dict(in_map) for _ in range(N_CORES)]
    res = run_bass_kernel_spmd(
        nc,
        in_maps,
        core_ids=list(range(N_CORES)),
        trace=bool(os.environ.get("BASS_TRACE")),
    )
    last_results = res
    return np.concatenate([r["out"] for r in res.results], axis=1)
